# revision 11
# baseline (speedup 1.0000x reference)
"""Trainium2 Bass kernel for nn_ConvEnhanced (conv+sigmoid mean / quantum sin^2 mean).

Math:
  classical = mean(sigmoid(conv2d(x, W) + b))           over [32,64,382,382]
  quantum   = mean(win3x3(sin^2(pi*x/2))) / 9           over [32,3,382,382]
  out = 0.5*classical + 0.5*quantum

Strategy (8 cores, batch-sharded, 4 images/core):
  - Classical: conv as matmul with dual block-diagonal weights.
    lhsT [54,128]: rows (i,c,dy,dx) i in {imgA,imgB}; cols = 2x64 out-chans.
    Two weight blocks live at PE rows 0-53 and 64-117 simultaneously
    (tile_position row 0/64), so no LDWEIGHTS churn between matmuls.
    rhs im2col tiles are built by a single 108-partition DMA whose
    per-partition reads are contiguous runs (full 384-wide rows; the
    (dy,dx) shift only moves the start offset).
    Sigmoid+bias+row-sum fused in one ACT op per 4 matmuls (accum_out).
  - Quantum: weighted sum is separable: sum_{i,j} wh(i)*ww(j)*s[i,j].
    On-chip: s = sin^2(pi/2 * m), m = x - 2*int(x*0.5) (range reduction,
    valid under trunc or RNE semantics), then PE matvec wh^T @ s -> [1,384]
    accumulated in PSUM; host applies ww.
  - Host combines per-core partial sums (stats [128,191] f32, qv [1,384]).
"""

import math
from contextlib import ExitStack

import ml_dtypes
import numpy as np

# ---- problem constants (hardcoded) ----
B, C, H, W_ = 32, 3, 384, 384
OC, KK = 64, 3
OH = OW = H - KK + 1  # 382
NCORES = 8
IPC = B // NCORES          # images per core = 4
ICC = IPC * C              # (img, ch) tiles per core = 12
IMG_CH = H * W_            # 147456 elements per (img, ch)
RC = 32                    # output rows per im2col DMA round
NGROUPS = (2 * OH) // 4    # 191 ACT groups per core (764 matmuls / 4)

_CACHE = {}
LAST_RESULTS = None  # BassKernelResults of the most recent run (for test.py)


def _build():
    import concourse.bacc as bacc
    import concourse.bass as bass
    import concourse.tile as tile
    from concourse import mybir
    from concourse.tile import add_dep_helper

    f32 = mybir.dt.float32
    bf16 = mybir.dt.bfloat16
    i32 = mybir.dt.int32
    Act = mybir.ActivationFunctionType
    Alu = mybir.AluOpType

    nc = bacc.Bacc("TRN2", target_bir_lowering=False, debug=False,
                   num_devices=NCORES)

    x_in = nc.dram_tensor("x", [ICC, 128, 1152], f32, kind="ExternalInput")
    w_in = nc.dram_tensor("wmat", [128, 128], bf16, kind="ExternalInput")
    b_in = nc.dram_tensor("bvec", [128, 1], f32, kind="ExternalInput")
    wh_in = nc.dram_tensor("whm", [128, 3], f32, kind="ExternalInput")
    st_o = nc.dram_tensor("stats", [128, NGROUPS], f32, kind="ExternalOutput")
    qv_o = nc.dram_tensor("qv", [1, 384], f32, kind="ExternalOutput")

    with tile.TileContext(nc) as tc, ExitStack() as ctx:
        singles = ctx.enter_context(tc.tile_pool(name="singles", bufs=1))
        dpool = ctx.enter_context(tc.tile_pool(name="dram", bufs=1, space="DRAM"))

        w_sb = singles.tile([128, 128], bf16)
        nc.sync.dma_start(w_sb[:], w_in.ap())
        b_sb = singles.tile([128, 1], f32)
        nc.sync.dma_start(b_sb[:], b_in.ap())
        wh_sb = singles.tile([128, 3], f32)
        nc.sync.dma_start(wh_sb[:], wh_in.ap())
        stats = singles.tile([128, NGROUPS], f32)
        dummy = singles.tile([128, 4, 382], f32)
        qsb = singles.tile([1, 384], f32)
        zb = singles.tile([128, 1], f32)
        nc.vector.memset(zb[:], 0.0)

        # bf16 copy of x, flat [img, ch, row, col], padded for dx-overrun reads
        xbf = dpool.tile([ICC * IMG_CH + 768], bf16)
        xbf_t = xbf[:].tensor
        zpad = singles.tile([1, 768], bf16)
        nc.vector.memset(zpad[:], 0.0)
        nc.sync.dma_start(
            xbf[ICC * IMG_CH:ICC * IMG_CH + 768].rearrange("(p f) -> p f", p=1),
            zpad[:])

        last_sin = None
        first_sig = None

        p0 = ctx.enter_context(tc.tile_pool(name="p0", bufs=3))
        rp = ctx.enter_context(tc.tile_pool(name="rhs", bufs=3))
        pp = ctx.enter_context(tc.tile_pool(name="cpsum", bufs=2, space="PSUM"))

        # ---------------- phase 0: quantum path + bf16 cast ----------------
        if True:
            # quantum accumulator borrows a slot of the classical PSUM pool;
            # it is released (end of phase 0) before the second classical
            # tile needs the slot.
            qp = pp.tile([1, 384], f32, tag="ps")
            for ic in range(ICC):
                xt = p0.tile([128, 1152], f32, tag="xt")
                nc.sync.dma_start(xt[:], x_in.ap()[ic])
                # bf16 cast + store for phase 1
                xb = p0.tile([128, 1152], bf16, tag="xb")
                nc.vector.tensor_copy(xb[:], xt[:])
                nc.sync.dma_start(
                    xbf[ic * IMG_CH:(ic + 1) * IMG_CH].rearrange(
                        "(p f) -> p f", p=128),
                    xb[:])
                # range reduction: m = x - 2*int(x*0.5)
                ri = p0.tile([128, 1152], i32, tag="ri")
                nc.vector.tensor_scalar(ri[:], xt[:], 0.5, None, Alu.mult)
                mt = p0.tile([128, 1152], f32, tag="mt")
                nc.vector.scalar_tensor_tensor(
                    mt[:], ri[:], -2.0, xt[:], Alu.mult, Alu.add)
                # s = sin(pi/2 * m); q = s*s
                st_t = p0.tile([128, 1152], f32, tag="st")
                ins = nc.scalar.activation(st_t[:], mt[:], Act.Sin,
                                           bias=zb[:, 0:1],
                                           scale=math.pi / 2)
                last_sin = ins
                qt = p0.tile([128, 1152], f32, tag="qt")
                nc.vector.tensor_mul(qt[:], st_t[:], st_t[:])
                # accumulate wh^T @ q into [1, 384]
                for t in range(3):
                    nc.tensor.matmul(
                        qp[:, :],
                        wh_sb[:, t:t + 1],
                        qt[:, 384 * t:384 * (t + 1)],
                        start=(ic == 0 and t == 0),
                        stop=(ic == ICC - 1 and t == 2))
            nc.vector.tensor_copy(qsb[:], qp[:])
            nc.sync.dma_start(qv_o.ap(), qsb[:])

        # ---------------- phase 1: conv + sigmoid + row-sums ----------------
        if True:
            g = 0
            mm_k = 0
            psum = None
            for r0 in range(0, OH, RC):
                rc = min(RC, OH - r0)
                rt = rp.tile([128, rc * 384], bf16, tag="rt")
                # 6 DMAs (2 blocks x 3 dy): partition q = 64b+18dy+9i+3c+dx
                # reads a contiguous rc*384 run of image (2b+i) channel c
                # from row r0+dy, col dx. Runs pair up in traversal order:
                # dest (18, F) <-> src (6, 3, F).
                for blk in (0, 1):
                    for dy in range(3):
                        dest = rt[64 * blk + 18 * dy:64 * blk + 18 * dy + 18, :]
                        src = bass.AP(
                            tensor=xbf_t,
                            offset=blk * 6 * IMG_CH + (r0 + dy) * 384,
                            ap=[[IMG_CH, 6], [1, 3], [1, rc * 384]])
                        nc.sync.dma_start(dest, src)
                for blk in (0, 1):
                    bp = 64 * blk
                    for r in range(rc):
                        if mm_k == 0:
                            psum = pp.tile([128, 2048], f32, tag="ps")
                        nc.tensor.matmul(
                            psum[:, 512 * mm_k:512 * mm_k + 382],
                            w_sb[bp:bp + 54, :],
                            rt[bp:bp + 54, r * 384:r * 384 + 382],
                            start=True, stop=True)
                        mm_k += 1
                        if mm_k == 4:
                            act_in = psum[:].rearrange(
                                "p (k c) -> p k c", k=4)[:, :, 0:382]
                            ins = nc.scalar.activation(
                                dummy[:], act_in, Act.Sigmoid,
                                bias=b_sb[:, 0:1], scale=1.0,
                                accum_out=stats[:, g:g + 1])
                            if first_sig is None:
                                first_sig = ins
                            g += 1
                            mm_k = 0
            assert g == NGROUPS and mm_k == 0
            nc.sync.dma_start(st_o.ap(), stats[:])

        # keep ACT ops phase-ordered (one table-set switch, not many)
        if first_sig is not None and last_sin is not None:
            add_dep_helper(first_sig.ins, last_sin.ins,
                           reason="sin table-set before sigmoid table-set")

    nc.compile()
    return nc


def _prep_host(W, b):
    # lhsT row order within each 64-block: q = 18*dy + 9*i + 3*c + dx
    wmat = np.zeros((128, 128), dtype=np.float32)
    for base in (0, 64):
        for dy in range(3):
            for i in range(2):
                for c in range(3):
                    for dx in range(3):
                        q = 18 * dy + 9 * i + 3 * c + dx
                        wmat[base + q, 64 * i:64 * i + OC] = W[:, c, dy, dx]
    wmat = wmat.astype(ml_dtypes.bfloat16)
    bvec = np.concatenate([b, b]).reshape(128, 1).astype(np.float32)
    i = np.arange(H)
    wvec = (np.minimum(i, OH - 1) - np.maximum(i - (KK - 1), 0) + 1)
    whm = wvec.astype(np.float32).reshape(128, 3)
    return wmat, bvec, whm, wvec.astype(np.float64)


def kernel(x, W, b):
    global LAST_RESULTS
    from concourse.bass_utils import run_bass_kernel_spmd

    if "nc" not in _CACHE:
        _CACHE["nc"] = _build()
    nc = _CACHE["nc"]

    x = np.ascontiguousarray(np.asarray(x, dtype=np.float32))
    wmat, bvec, whm, wvec = _prep_host(np.asarray(W, np.float32),
                                       np.asarray(b, np.float32))
    in_maps = []
    for cid in range(NCORES):
        xs = np.ascontiguousarray(
            x[IPC * cid:IPC * (cid + 1)]).reshape(ICC, 128, 1152)
        in_maps.append({"x": xs, "wmat": wmat, "bvec": bvec, "whm": whm})

    import os
    trace = bool(int(os.environ.get("KERNEL_TRACE", "0")))
    res = run_bass_kernel_spmd(nc, in_maps, core_ids=list(range(NCORES)),
                               trace=trace)
    LAST_RESULTS = res

    cl = 0.0
    qv = np.zeros(384, np.float64)
    for r in res.results:
        cl += r["stats"].astype(np.float64).sum()
        qv += r["qv"][0].astype(np.float64)
    classical_mean = cl / (B * OC * OH * OW)
    quantum_mean = float((qv * wvec).sum()) / (B * C * OH * OW * KK * KK)
    return np.float32(0.5 * classical_mean + 0.5 * quantum_mean)
